# revision 4
# baseline (speedup 1.0000x reference)
"""Trainium2 Bass kernel for nn_Encoder_81595788689580.

Attention-gated GRU encoder: per time step
    w1 = h @ attn1_W.T + attn1_b
    w2 = x_t @ attn2_W.T + attn2_b
    v  = tanh(w1 + w2) @ attn3_W.T + attn3_b
    alpha = softmax(v, axis=feature)
    wx = x_t * alpha
    GRU cell (r, z, n) -> h_new
Output: [B, T, H] hidden states.

Strategy (8 NeuronCores, data-parallel over batch; 512 rows/core run as
2 pipelined chunks of 256):
  - transposed layout: features on partitions, batch on the free dim;
    matmuls are weights-stationary, all biases ride inside the matmuls
    as ones-rows in the zero-padded contraction tiles (no ACT bias ops).
  - fp8(e4m3) DoubleRow matmuls: pairs of 128-row contraction tiles are
    packed [128, 2, N] and processed in one PE pass (2x vs fp16).  The
    moving tile mh = [x0 x1 x2|h0 h1 pad] makes (x0,x1),(x2,h0),(h1,z)
    adjacent DR pairs for the attention + r/z-gate contractions.
  - precision: everything fp8 except the n-gate recurrent path
    (W_hh,n . h in fp16 with 0.5*S folded into the weights) and the
    n-gate input bias (fp16 per-partition scalar in a DVE op); wx is
    stored *S (S=16) to stay in fp8 normal range, descaled by the ACT
    `scale` at each gate activation.  Simulated rel err ~2e-3.
  - softmax denominator via a fp16 ones-matmul (value 1/S so the
    reciprocal directly yields the S-scaled alpha normalizer).
  - sigmoid as 0.5*tanh(x/2)+0.5 (one ACT table set: exp+tanh).
  - engine balance per chunk-step: PE ~40 col-units, ACT 5 ops (tanh,
    exp, gates, n, h->fp8 copy), DVE 9 ops, Pool (gpsimd) does the
    rinv cast and the wx = x*alpha*S fp8 product.
"""

import numpy as np

B, T, I, H = 4096, 24, 320, 256
NCORES = 8
BS = B // NCORES          # 512 rows per core
NCHUNK = 2
CB = BS // NCHUNK         # 256 batch columns per chunk
IP = 384                  # I padded to 3*128
KI = IP // 128            # 3 feature blocks
KH = H // 128             # 2 hidden blocks
S = 16.0                  # wx / gate-psum scale (fp8 range)

_STATE = {}


def _build(t_steps=T):
    import concourse.bass as bass
    import concourse.tile as tile
    from concourse import bacc, mybir

    f32 = mybir.dt.float32
    f16 = mybir.dt.float16
    f8 = mybir.dt.float8e4
    AF = mybir.ActivationFunctionType
    OP = mybir.AluOpType
    DR = mybir.MatmulPerfMode.DoubleRow

    nc = bacc.Bacc("TRN2", target_bir_lowering=False, debug=False,
                   num_devices=NCORES)

    x8d = nc.dram_tensor("x8", [t_steps, NCHUNK, 128, KI, CB], f8,
                         kind="ExternalInput").ap()
    x16d = nc.dram_tensor("x16", [t_steps, NCHUNK, 128, KI, CB], f16,
                          kind="ExternalInput").ap()
    h08d = nc.dram_tensor("h08", [NCHUNK, 128, KH, CB], f8,
                          kind="ExternalInput").ap()
    h016d = nc.dram_tensor("h016", [NCHUNK, 128, KH, CB], f16,
                           kind="ExternalInput").ap()
    wu8d = nc.dram_tensor("wu8", [128, 3, 2, IP], f8,
                          kind="ExternalInput").ap()
    wv8d = nc.dram_tensor("wv8", [128, 2, 2, IP], f8,
                          kind="ExternalInput").ap()
    wrz8d = nc.dram_tensor("wrz8", [128, 3, 2, 512], f8,
                           kind="ExternalInput").ap()
    wp8d = nc.dram_tensor("wp8", [128, 2, 2, 256], f8,
                          kind="ExternalInput").ap()
    wt16d = nc.dram_tensor("wt16", [128, 3, 256], f16,
                           kind="ExternalInput").ap()
    ones16d = nc.dram_tensor("ones16", [128, 128], f16,
                             kind="ExternalInput").ap()
    bin16d = nc.dram_tensor("bin16", [128, 2], f16,
                            kind="ExternalInput").ap()
    uz8d = nc.dram_tensor("uz8", [128, CB], f8, kind="ExternalInput").ap()
    h2c16d = nc.dram_tensor("h2c16", [128, CB], f16,
                            kind="ExternalInput").ap()
    outd = nc.dram_tensor("outT", [t_steps, NCHUNK, 128, KH, CB], f16,
                          kind="ExternalOutput").ap()

    MH_BUFS = 4
    U_BUFS = 3
    H_BUFS = 4

    def ms(m):
        return slice(m * 128, (m + 1) * 128)

    with tile.TileContext(nc) as tc:
        with tc.tile_pool(name="const", bufs=1) as cp, \
             tc.tile_pool(name="mhp", bufs=1) as mp, \
             tc.tile_pool(name="wk", bufs=1) as wp, \
             tc.tile_pool(name="ps", bufs=1, space="PSUM") as pp:

            wu8 = cp.tile([128, 3, 2, IP], f8)
            wv8 = cp.tile([128, 2, 2, IP], f8)
            wrz8 = cp.tile([128, 3, 2, 512], f8)
            wp8 = cp.tile([128, 2, 2, 256], f8)
            wt16 = cp.tile([128, 3, 256], f16)
            ones16 = cp.tile([128, 128], f16)
            bin16 = cp.tile([128, 2], f16)
            for i, (dst, src) in enumerate([
                    (wu8, wu8d), (wv8, wv8d), (wrz8, wrz8d), (wp8, wp8d),
                    (wt16, wt16d), (ones16, ones16d), (bin16, bin16d)]):
                eng = nc.sync if i % 2 == 0 else nc.scalar
                eng.dma_start(out=dst[:], in_=src)

            ci_of = {}

            def mh_tile(t, c, n):
                # moving tile: slots 0-2 x8/wx8, 3-4 h8, 5 static zeros
                mt = mp.tile([128, 6, CB], f8, name=f"mh_{t}_{c}", tag="mh",
                             bufs=MH_BUFS, allow_tmpbuf=False)
                if n < MH_BUFS:
                    nc.gpsimd.memset(mt[:, 5, :], 0.0)
                return mt

            # step-0 moving tiles + h state
            mh = {}
            h16 = {}
            nalloc = 0
            for c in range(NCHUNK):
                mt = mh_tile(0, c, nalloc)
                nalloc += 1
                nc.sync.dma_start(out=mt[:, 0:3, :], in_=x8d[0, c])
                nc.sync.dma_start(out=mt[:, 3:5, :], in_=h08d[c])
                mh[c] = mt
                ht = wp.tile([128, 3, CB], f16, name=f"h16_i_{c}", tag="h16",
                             bufs=H_BUFS)
                nc.scalar.dma_start(out=ht[:, 0:2, :], in_=h016d[c])
                nc.scalar.dma_start(out=ht[:, 2, :], in_=h2c16d)
                h16[c] = ht

            x16 = {}
            for c in range(NCHUNK):
                xt = wp.tile([128, KI, CB], f16, name=f"x16_0_{c}", tag="x16",
                             bufs=4)
                nc.sync.dma_start(out=xt[:], in_=x16d[0, c])
                x16[c] = xt

            for t in range(t_steps):
                # prefetch next step's x (and allocate next mh so this
                # step's tail can deposit h8 into it)
                mh_nxt = {}
                if t + 1 < t_steps:
                    for c in range(NCHUNK):
                        mt = mh_tile(t + 1, c, nalloc)
                        nalloc += 1
                        nc.sync.dma_start(out=mt[:, 0:3, :],
                                          in_=x8d[t + 1, c])
                        mh_nxt[c] = mt
                        xt = wp.tile([128, KI, CB], f16,
                                     name=f"x16_{t + 1}_{c}", tag="x16",
                                     bufs=4)
                        nc.sync.dma_start(out=xt[:], in_=x16d[t + 1, c])
                        x16[NCHUNK + c] = xt

                st = [{} for _ in range(NCHUNK)]

                # ---- phase A: ps_u = attn2.x + attn1.h (+bu), tanh ----
                for c in range(NCHUNK):
                    att_u = pp.tile([128, 4, CB], f32, name=f"psu_{t}_{c}",
                                    tag="att", bufs=2)
                    for m in range(3):
                        for p in range(3):
                            nc.tensor.matmul(
                                att_u[:, m, :], wu8[:, p, :, ms(m)],
                                mh[c][:, 2 * p:2 * p + 2, :],
                                start=(p == 0), stop=(p == 2), perf_mode=DR)
                    u8 = wp.tile([128, 4, CB], f8, name=f"u8_{t}_{c}",
                                 tag="u8", bufs=U_BUFS)
                    st[c]["u8"] = u8
                    st[c]["att_u"] = att_u

                # uz slot (static ones-row) DMA'd once per fresh buffer
                for c in range(NCHUNK):
                    u8 = st[c]["u8"]
                    if t * NCHUNK + c < U_BUFS:
                        nc.scalar.dma_start(out=u8[:, 3, :], in_=uz8d)
                    nc.scalar.activation(u8[:, 0:3, :],
                                         st[c]["att_u"][:, 0:3, :], AF.Tanh)

                # ---- phase B: ps_v = attn3.u (+bv), exp ----
                for c in range(NCHUNK):
                    att_v = pp.tile([128, 4, CB], f32, name=f"psv_{t}_{c}",
                                    tag="att", bufs=2)
                    u8 = st[c]["u8"]
                    for m in range(3):
                        for p in range(2):
                            nc.tensor.matmul(
                                att_v[:, m, :], wv8[:, p, :, ms(m)],
                                u8[:, 2 * p:2 * p + 2, :],
                                start=(p == 0), stop=(p == 1), perf_mode=DR)
                    ev = wp.tile([128, 3, CB], f16, name=f"ev_{t}_{c}",
                                 tag="ev", bufs=3)
                    nc.scalar.activation(ev[:], att_v[:, 0:3, :], AF.Exp)
                    st[c]["ev"] = ev
                    st[c]["att_v"] = att_v

                # ---- phase C: den, rinv, evs, wx ----
                for c in range(NCHUNK):
                    ev = st[c]["ev"]
                    att_v = st[c]["att_v"]
                    for k in range(3):
                        nc.tensor.matmul(att_v[:, 3, :], ones16[:],
                                         ev[:, k, :], start=(k == 0),
                                         stop=(k == 2))
                    rinv = wp.tile([128, CB], f32, name=f"rinv_{t}_{c}",
                                   tag="rinv", bufs=2)
                    nc.vector.reciprocal_approx_fast(rinv[:], att_v[:, 3, :])
                    rinv16 = wp.tile([128, CB], f16, name=f"rinv16_{t}_{c}",
                                     tag="rinv16", bufs=2)
                    nc.gpsimd.tensor_copy(rinv16[:], rinv[:])
                    evs = wp.tile([128, 3, CB], f16, name=f"evs_{t}_{c}",
                                  tag="evs", bufs=2)
                    _r = rinv16[:]
                    rrep = bass.AP(tensor=_r.tensor, offset=_r.offset,
                                   ap=[_r.ap[0], [0, 3], _r.ap[1]])
                    nc.vector.tensor_tensor(out=evs[:], in0=ev[:], in1=rrep,
                                            op=OP.mult)
                    # wx8 = x16 * evs  (= S * x * alpha), into mh slots 0-2;
                    # slot2 rows 64.. keep the DMA'd bias row (row64 = 1.0)
                    xc = x16[c]
                    nc.gpsimd.tensor_tensor(
                        out=mh[c][:, 0:2, :], in0=xc[:, 0:2, :],
                        in1=evs[:, 0:2, :], op=OP.mult)
                    nc.gpsimd.tensor_tensor(
                        out=mh[c][0:64, 2, :], in0=xc[0:64, 2, :],
                        in1=evs[0:64, 2, :], op=OP.mult)

                # ---- phase D: gates + GRU tail ----
                for c in range(NCHUNK):
                    gps = pp.tile([128, 4, CB], f32, name=f"gps_{t}_{c}",
                                  tag="gps", bufs=1)
                    for mb in range(4):
                        for p in range(3):
                            nc.tensor.matmul(
                                gps[:, mb, :], wrz8[:, p, :, ms(mb)],
                                mh[c][:, 2 * p:2 * p + 2, :],
                                start=(p == 0), stop=(p == 2), perf_mode=DR)
                    pt = pp.tile([128, 4, CB], f32, name=f"pt_{t}_{c}",
                                 tag="pt", bufs=1)
                    for mb in range(2):
                        for p in range(2):
                            nc.tensor.matmul(
                                pt[:, mb, :], wp8[:, p, :, ms(mb)],
                                mh[c][:, 2 * p:2 * p + 2, :],
                                start=(p == 0), stop=(p == 1), perf_mode=DR)
                    hprev = h16[c]
                    for mb in range(2):
                        for k in range(3):
                            nc.tensor.matmul(
                                pt[:, 2 + mb, :], wt16[:, k, ms(mb)],
                                hprev[:, k, :], start=(k == 0),
                                stop=(k == 2))

                    g16 = wp.tile([128, 4, CB], f16, name=f"g_{t}_{c}",
                                  tag="g", bufs=2)
                    nc.scalar.activation(g16[:], gps[:], AF.Tanh,
                                         scale=0.5 / S)
                    # rt = (g+1)*t1h = S * r * (Whh_n.h + b_hh_n)
                    rt = wp.tile([128, 2, CB], f16, name=f"rt_{t}_{c}",
                                 tag="rt", bufs=2)
                    nc.vector.scalar_tensor_tensor(
                        rt[:], g16[:, 0:2, :], 1.0, pt[:, 2:4, :],
                        OP.add, OP.mult)
                    s2 = wp.tile([128, 2, CB], f16, name=f"s2_{t}_{c}",
                                 tag="s2", bufs=2)
                    for mb in range(2):
                        nc.vector.scalar_tensor_tensor(
                            s2[:, mb, :], pt[:, mb, :],
                            bin16[:, mb:mb + 1], rt[:, mb, :],
                            OP.add, OP.add)
                    n16 = wp.tile([128, 2, CB], f16, name=f"n_{t}_{c}",
                                  tag="n", bufs=2)
                    nc.scalar.activation(n16[:], s2[:], AF.Tanh,
                                         scale=1.0 / S)
                    q16 = wp.tile([128, 2, CB], f16, name=f"q_{t}_{c}",
                                  tag="q", bufs=2)
                    nc.vector.tensor_tensor(out=q16[:],
                                            in0=hprev[:, 0:2, :],
                                            in1=n16[:], op=OP.subtract)
                    zq = wp.tile([128, 2, CB], f16, name=f"zq_{t}_{c}",
                                 tag="zq", bufs=2)
                    nc.vector.scalar_tensor_tensor(
                        zq[:], g16[:, 2:4, :], 0.5, q16[:], OP.mult, OP.mult)
                    u2 = wp.tile([128, 2, CB], f16, name=f"u2_{t}_{c}",
                                 tag="u2", bufs=2)
                    nc.vector.scalar_tensor_tensor(
                        u2[:], q16[:], 0.5, n16[:], OP.mult, OP.add)
                    hnew = wp.tile([128, 3, CB], f16, name=f"h16_{t}_{c}",
                                   tag="h16", bufs=H_BUFS)
                    halloc = NCHUNK + t * NCHUNK + c
                    if halloc < H_BUFS:
                        nc.scalar.dma_start(out=hnew[:, 2, :], in_=h2c16d)
                    nc.vector.tensor_tensor(out=hnew[:, 0:2, :], in0=zq[:],
                                            in1=u2[:], op=OP.add)
                    h16[c] = hnew
                    if t + 1 < t_steps:
                        nc.scalar.activation(mh_nxt[c][:, 3:5, :],
                                             hnew[:, 0:2, :], AF.Copy)
                    nc.sync.dma_start(out=outd[t, c], in_=hnew[:, 0:2, :])

                for c in range(NCHUNK):
                    if t + 1 < t_steps:
                        mh[c] = mh_nxt[c]
                        x16[c] = x16[NCHUNK + c]

    nc.compile()
    return nc


# ---------------- host-side data prep ----------------

def _prep_core_inputs(x, h0, attn1_W, attn1_b, attn2_W, attn2_b, attn3_W,
                      attn3_b, W_ih, b_ih, W_hh, b_hh, t_steps=T):
    import ml_dtypes
    f4 = np.float32
    f16n = np.float16
    f8n = ml_dtypes.float8_e4m3

    x = np.asarray(x, f4)
    h0 = np.asarray(h0, f4)

    A1T = np.zeros((H, IP), f4)
    A1T[:, :I] = np.asarray(attn1_W, f4).T
    A2T = np.zeros((IP, IP), f4)
    A2T[:I, :I] = np.asarray(attn2_W, f4).T
    A3T = np.zeros((IP, IP), f4)
    A3T[:I, :I] = np.asarray(attn3_W, f4).T
    WihT = np.zeros((IP, 3 * H), f4)
    WihT[:I, :] = np.asarray(W_ih, f4).T
    WhhT = np.asarray(W_hh, f4).T                      # [256, 768]
    bu = np.zeros(IP, f4)
    bu[:I] = np.asarray(attn1_b, f4) + np.asarray(attn2_b, f4)
    bv = np.full(IP, -448.0, f4)
    bv[:I] = np.asarray(attn3_b, f4)
    brz = (np.asarray(b_ih, f4) + np.asarray(b_hh, f4))[:2 * H]
    bhn = np.asarray(b_hh, f4)[2 * H:]
    bin_ = np.asarray(b_ih, f4)[2 * H:]

    # wu8: pairs over mh slots (x0,x1),(x2,h0),(h1,zero); out cols = IP
    wu = np.zeros((3, 2, 128, IP), f4)
    wu[0, 0] = A2T[0:128]
    wu[0, 1] = A2T[128:256]
    wu[1, 0] = A2T[256:384]
    wu[1, 0, 64, :] = bu                              # ones-row bias
    wu[1, 1] = A1T[0:128].reshape(128, IP)
    wu[2, 0] = A1T[128:256]
    wu8 = np.ascontiguousarray(wu.transpose(2, 0, 1, 3)).astype(f8n)

    # wv8: pairs (u0,u1),(u2,uz); uz row0 carries bv
    wv = np.zeros((2, 2, 128, IP), f4)
    wv[0, 0] = A3T[0:128]
    wv[0, 1] = A3T[128:256]
    wv[1, 0] = A3T[256:384]
    wv[1, 1, 0, :] = bv
    wv8 = np.ascontiguousarray(wv.transpose(2, 0, 1, 3)).astype(f8n)

    # wrz8: out cols = r(256) z(256); h-side weights carry S
    wrz = np.zeros((3, 2, 128, 512), f4)
    wrz[0, 0] = WihT[0:128, :512]
    wrz[0, 1] = WihT[128:256, :512]
    wrz[1, 0] = WihT[256:384, :512]
    wrz[1, 0, 64, :] = S * brz
    wrz[1, 1] = S * WhhT[0:128, :512]
    wrz[2, 0] = S * WhhT[128:256, :512]
    wrz8 = np.ascontiguousarray(wrz.transpose(2, 0, 1, 3)).astype(f8n)

    # wp8: i_n path, pairs (x0,x1),(x2,zero-h0); no bias row (fp16 STT)
    wpp = np.zeros((2, 2, 128, 256), f4)
    wpp[0, 0] = WihT[0:128, 512:]
    wpp[0, 1] = WihT[128:256, 512:]
    wpp[1, 0] = WihT[256:384, 512:]
    wp8 = np.ascontiguousarray(wpp.transpose(2, 0, 1, 3)).astype(f8n)

    # wt16: t1h = 0.5*S*(Whh_n . h + b_hh_n); k2 = fp16 ones-row bias
    wt = np.zeros((3, 128, 256), f4)
    wt[0] = 0.5 * S * WhhT[0:128, 512:]
    wt[1] = 0.5 * S * WhhT[128:256, 512:]
    wt[2, 0, :] = 0.5 * S * bhn
    wt16 = np.ascontiguousarray(wt.transpose(1, 0, 2)).astype(f16n)

    ones16 = np.full((128, 128), 1.0 / S, f16n)
    bin16 = np.ascontiguousarray((S * bin_).reshape(2, 128).T).astype(f16n)
    uz8 = np.zeros((128, CB), f8n)
    uz8[0, :] = 1.0
    h2c16 = np.zeros((128, CB), f16n)
    h2c16[0, :] = 1.0

    # x: pad to IP, plant the ones-row at feature 320 (slot2 row64)
    xp = np.zeros((B, t_steps, IP), f4)
    xp[:, :, :I] = x[:, :t_steps, :]
    xp[:, :, I] = 1.0
    # [NC, CHUNK, CB, T, KI, 128] -> [NC, T, CHUNK, 128, KI, CB]
    xr = xp.reshape(NCORES, NCHUNK, CB, t_steps, KI, 128)
    xr = xr.transpose(0, 3, 1, 5, 4, 2)
    x8 = np.ascontiguousarray(xr).astype(f8n)
    x16 = np.ascontiguousarray(xr).astype(f16n)

    h0r = h0.reshape(NCORES, NCHUNK, CB, KH, 128).transpose(0, 1, 4, 3, 2)
    h08 = np.ascontiguousarray(h0r).astype(f8n)
    h016 = np.ascontiguousarray(h0r).astype(f16n)

    shared = dict(wu8=wu8, wv8=wv8, wrz8=wrz8, wp8=wp8, wt16=wt16,
                  ones16=ones16, bin16=bin16, uz8=uz8, h2c16=h2c16)
    in_maps = []
    for c in range(NCORES):
        m = dict(shared)
        m["x8"] = x8[c]
        m["x16"] = x16[c]
        m["h08"] = h08[c]
        m["h016"] = h016[c]
        in_maps.append(m)
    return in_maps


def _gather(results, t_steps=T):
    outs = []
    for c in range(NCORES):
        o = np.asarray(results[c]["outT"], np.float32)
        # [T, CHUNK, 128, KH, CB] -> [CHUNK, CB, T, KH, 128] -> [BS, T, H]
        o = o.transpose(1, 4, 0, 3, 2).reshape(BS, t_steps, H)
        outs.append(o)
    return np.ascontiguousarray(np.concatenate(outs, axis=0))


def _get_nc(t_steps=T):
    key = ("nc", t_steps)
    if key not in _STATE:
        _STATE[key] = _build(t_steps)
    return _STATE[key]


def run(inputs, trace=False, t_steps=T):
    from concourse.bass_utils import run_bass_kernel_spmd
    nc = _get_nc(t_steps)
    in_maps = _prep_core_inputs(t_steps=t_steps, **inputs)
    res = run_bass_kernel_spmd(nc, in_maps, list(range(NCORES)), trace=trace)
    return _gather(res.results, t_steps), res


def kernel(**inputs):
    out, _ = run(inputs, trace=False)
    return out


# revision 7
# speedup vs baseline: 1.5003x; 1.5003x over previous
"""Trainium2 Bass kernel for nn_Encoder_81595788689580.

Attention-gated GRU encoder: per time step
    w1 = h @ attn1_W.T + attn1_b
    w2 = x_t @ attn2_W.T + attn2_b
    v  = tanh(w1 + w2) @ attn3_W.T + attn3_b
    alpha = softmax(v, axis=feature)
    wx = x_t * alpha
    GRU cell (r, z, n) -> h_new
Output: [B, T, H] hidden states.

Strategy (8 NeuronCores, data-parallel over batch; 512 rows/core run as
2 pipelined chunks of 256):
  - transposed layout: features on partitions, batch on the free dim;
    weights-stationary matmuls, biases ride inside the matmuls as
    ones-rows planted in the zero-padding (x feature 320 = 1.0, and an
    fp16 ones-row slot in the h state tile).
  - x-side contractions in fp8(e4m3): the (x0,x1) 256-row pair runs as
    one DoubleRow matmul (2x), x2 as a plain fp8 matmul carrying the
    bias row; h-side contractions stay fp16 (exact recurrent path,
    keeps PE busy enough to hold the 2.4GHz p-state, and avoids an
    h->fp8 cast on the critical path).
  - wx is stored *S (S=16) in fp8 (normal range), descaled by the ACT
    `scale` at each gate activation; the r/z h-side weights carry S,
    the n-path h-side weights carry 0.5*S (folding the sigmoid scale).
  - precision: n-gate path fp16 end-to-end; sim rel err ~2e-3.
  - chain shortening: h-only matmuls (t1h, bias rows) fill the PE
    while ACT runs; the gate ACT is split r-first/z-second so the
    n-gate chain starts after only the r blocks; the z-path products
    (zz, w1z on DVE, bzh on gpsimd) run off the critical chain.
  - softmax denominator: fp16 ones-matmul (value 1/S) so reciprocal
    yields the S-scaled normalizer directly (read as f32 by the DVE
    alpha multiply; no cast hop).
"""

import numpy as np

B, T, I, H = 4096, 24, 320, 256
NCORES = 8
BS = B // NCORES          # 512 rows per core
NCHUNK = 2
CB = BS // NCHUNK         # 256 batch columns per chunk
IP = 384                  # I padded to 3*128
KI = IP // 128            # 3 feature blocks
KH = H // 128             # 2 hidden blocks
S = 16.0                  # wx / gate-psum scale (fp8 range)

_STATE = {}


def _build(t_steps=T):
    import concourse.bass as bass
    import concourse.tile as tile
    from concourse import bacc, mybir

    f32 = mybir.dt.float32
    f16 = mybir.dt.float16
    f8 = mybir.dt.float8e4
    AF = mybir.ActivationFunctionType
    OP = mybir.AluOpType
    DR = mybir.MatmulPerfMode.DoubleRow

    nc = bacc.Bacc("TRN2", target_bir_lowering=False, debug=False,
                   num_devices=NCORES)

    x8d = nc.dram_tensor("x8", [t_steps, NCHUNK, 128, KI, CB], f8,
                         kind="ExternalInput").ap()
    x16d = nc.dram_tensor("x16", [t_steps, NCHUNK, 128, KI, CB], f16,
                          kind="ExternalInput").ap()
    h016d = nc.dram_tensor("h016", [NCHUNK, 128, KH, CB], f16,
                           kind="ExternalInput").ap()
    # fp8 stationaries: [pair(x0x1) as DR | x2 plain] per consumer
    wu8d = nc.dram_tensor("wu8", [128, 3, IP], f8,
                          kind="ExternalInput").ap()
    wv8d = nc.dram_tensor("wv8", [128, 2, 2, IP], f8,
                          kind="ExternalInput").ap()
    wrz8d = nc.dram_tensor("wrz8", [128, 3, 512], f8,
                           kind="ExternalInput").ap()
    wp8d = nc.dram_tensor("wp8", [128, 3, 256], f8,
                          kind="ExternalInput").ap()
    # fp16 stationaries (h-side + biases)
    wu16d = nc.dram_tensor("wu16", [128, 2, IP], f16,
                           kind="ExternalInput").ap()
    wrz16d = nc.dram_tensor("wrz16", [128, 2, 512], f16,
                            kind="ExternalInput").ap()
    wt16d = nc.dram_tensor("wt16", [128, 3, 256], f16,
                           kind="ExternalInput").ap()
    wbin16d = nc.dram_tensor("wbin16", [128, 256], f16,
                             kind="ExternalInput").ap()
    ones16d = nc.dram_tensor("ones16", [128, 128], f16,
                             kind="ExternalInput").ap()
    uz8d = nc.dram_tensor("uz8", [128, CB], f8, kind="ExternalInput").ap()
    h2c16d = nc.dram_tensor("h2c16", [128, CB], f16,
                            kind="ExternalInput").ap()
    outd = nc.dram_tensor("outT", [t_steps, NCHUNK, 128, KH, CB], f16,
                          kind="ExternalOutput").ap()

    MH_BUFS = 4
    U_BUFS = 3
    H_BUFS = 4

    def ms(m):
        return slice(m * 128, (m + 1) * 128)

    with tile.TileContext(nc) as tc:
        with tc.tile_pool(name="const", bufs=1) as cp, \
             tc.tile_pool(name="mhp", bufs=1) as mp, \
             tc.tile_pool(name="wk", bufs=1) as wp, \
             tc.tile_pool(name="ps", bufs=1, space="PSUM") as pp:

            wu8 = cp.tile([128, 3, IP], f8)
            wv8 = cp.tile([128, 2, 2, IP], f8)
            wrz8 = cp.tile([128, 3, 512], f8)
            wp8 = cp.tile([128, 3, 256], f8)
            wu16 = cp.tile([128, 2, IP], f16)
            wrz16 = cp.tile([128, 2, 512], f16)
            wt16 = cp.tile([128, 3, 256], f16)
            wbin16 = cp.tile([128, 256], f16)
            ones16 = cp.tile([128, 128], f16)
            for i, (dst, src) in enumerate([
                    (wu8, wu8d), (wv8, wv8d), (wrz8, wrz8d), (wp8, wp8d),
                    (wu16, wu16d), (wrz16, wrz16d), (wt16, wt16d),
                    (wbin16, wbin16d), (ones16, ones16d)]):
                eng = nc.sync if i % 2 == 0 else nc.scalar
                eng.dma_start(out=dst[:], in_=src)

            # step-0 moving tiles + h state
            mh = {}
            h16 = {}
            x16 = {}
            for c in range(NCHUNK):
                mt = mp.tile([128, KI, CB], f8, name=f"mh_0_{c}", tag="mh",
                             bufs=MH_BUFS)
                nc.sync.dma_start(out=mt[:], in_=x8d[0, c])
                mh[c] = mt
                ht = wp.tile([128, 3, CB], f16, name=f"h16_i_{c}", tag="h16",
                             bufs=H_BUFS)
                nc.scalar.dma_start(out=ht[:, 0:2, :], in_=h016d[c])
                nc.scalar.dma_start(out=ht[:, 2, :], in_=h2c16d)
                h16[c] = ht
                xt = wp.tile([128, KI, CB], f16, name=f"x16_0_{c}", tag="x16",
                             bufs=4)
                nc.sync.dma_start(out=xt[:], in_=x16d[0, c])
                x16[c] = xt

            for t in range(t_steps):
                mh_nxt = {}
                if t + 1 < t_steps:
                    for c in range(NCHUNK):
                        mt = mp.tile([128, KI, CB], f8, name=f"mh_{t + 1}_{c}",
                                     tag="mh", bufs=MH_BUFS)
                        nc.sync.dma_start(out=mt[:], in_=x8d[t + 1, c])
                        mh_nxt[c] = mt
                        xt = wp.tile([128, KI, CB], f16,
                                     name=f"x16_{t + 1}_{c}", tag="x16",
                                     bufs=4)
                        nc.sync.dma_start(out=xt[:], in_=x16d[t + 1, c])
                        x16[NCHUNK + c] = xt

                st = [{} for _ in range(NCHUNK)]

                # ---- phase A: ps_u = attn2.x + attn1.h (+bu), tanh ----
                for c in range(NCHUNK):
                    att_u = pp.tile([128, 4, CB], f32, name=f"psu_{t}_{c}",
                                    tag="att", bufs=2)
                    for m in range(3):
                        nc.tensor.matmul(att_u[:, m, :], wu8[:, 0:2, ms(m)],
                                         mh[c][:, 0:2, :], start=True,
                                         stop=False, perf_mode=DR)
                        nc.tensor.matmul(att_u[:, m, :], wu8[:, 2, ms(m)],
                                         mh[c][:, 2, :], start=False,
                                         stop=False)
                        for k in range(2):
                            nc.tensor.matmul(att_u[:, m, :],
                                             wu16[:, k, ms(m)],
                                             h16[c][:, k, :], start=False,
                                             stop=(k == 1))
                    st[c]["att_u"] = att_u

                # t1h(c0) (h-only, fp16): fills PE while tanh/exp run
                def emit_t1h(c):
                    t1h = pp.tile([128, 2, CB], f32, name=f"t1h_{t}_{c}",
                                  tag="gp", bufs=4)
                    for mb in range(2):
                        for k in range(3):
                            nc.tensor.matmul(
                                t1h[:, mb, :], wt16[:, k, ms(mb)],
                                h16[c][:, k, :], start=(k == 0),
                                stop=(k == 2))
                    st[c]["t1h"] = t1h

                emit_t1h(0)

                for c in range(NCHUNK):
                    u8 = wp.tile([128, 4, CB], f8, name=f"u8_{t}_{c}",
                                 tag="u8", bufs=U_BUFS)
                    if t * NCHUNK + c < U_BUFS:
                        nc.scalar.dma_start(out=u8[:, 3, :], in_=uz8d)
                    nc.scalar.activation(u8[:, 0:3, :],
                                         st[c]["att_u"][:, 0:3, :], AF.Tanh)
                    st[c]["u8"] = u8

                # ---- phase B: ps_v = attn3.u (+bv), exp ----
                for c in range(NCHUNK):
                    att_v = pp.tile([128, 4, CB], f32, name=f"psv_{t}_{c}",
                                    tag="att", bufs=2)
                    u8 = st[c]["u8"]
                    for m in range(3):
                        for p in range(2):
                            nc.tensor.matmul(
                                att_v[:, m, :], wv8[:, p, :, ms(m)],
                                u8[:, 2 * p:2 * p + 2, :],
                                start=(p == 0), stop=(p == 1), perf_mode=DR)
                    ev = wp.tile([128, 3, CB], f16, name=f"ev_{t}_{c}",
                                 tag="ev", bufs=3)
                    nc.scalar.activation(ev[:], att_v[:, 0:3, :], AF.Exp)
                    st[c]["ev"] = ev
                    st[c]["att_v"] = att_v

                # ---- phase C: den, rinv, evs, wx ----
                for c in range(NCHUNK):
                    ev = st[c]["ev"]
                    att_v = st[c]["att_v"]
                    for k in range(3):
                        nc.tensor.matmul(att_v[:, 3, :], ones16[:],
                                         ev[:, k, :], start=(k == 0),
                                         stop=(k == 2))
                    rinv = wp.tile([128, CB], f32, name=f"rinv_{t}_{c}",
                                   tag="rinv", bufs=2)
                    nc.vector.reciprocal_approx_fast(rinv[:], att_v[:, 3, :])
                    evs = wp.tile([128, 3, CB], f16, name=f"evs_{t}_{c}",
                                  tag="evs", bufs=2)
                    _r = rinv[:]
                    rrep = bass.AP(tensor=_r.tensor, offset=_r.offset,
                                   ap=[_r.ap[0], [0, 3], _r.ap[1]])
                    nc.vector.tensor_tensor(out=evs[:], in0=ev[:], in1=rrep,
                                            op=OP.mult)
                    # wx8 = x16 * evs (= S*x*alpha) into mh slots 0-2 on
                    # gpsimd; slot2 rows 64.. keep the DMA'd row64 = 1.0
                    xc = x16[c]
                    nc.gpsimd.tensor_tensor(
                        out=mh[c][:, 0:2, :], in0=xc[:, 0:2, :],
                        in1=evs[:, 0:2, :], op=OP.mult)
                    nc.gpsimd.tensor_tensor(
                        out=mh[c][0:64, 2, :], in0=xc[0:64, 2, :],
                        in1=evs[0:64, 2, :], op=OP.mult)

                # ---- phase D: gates + GRU tail ----
                for c in range(NCHUNK):
                    hprev = h16[c]
                    if c == 1:
                        emit_t1h(1)
                    t1h = st[c]["t1h"]
                    rr = pp.tile([128, 2, CB], f32, name=f"rps_{t}_{c}",
                                 tag="gp", bufs=4)
                    inp = pp.tile([128, 2, CB], f32, name=f"inps_{t}_{c}",
                                  tag="gp", bufs=4)
                    zz_ps = pp.tile([128, 2, CB], f32, name=f"zps_{t}_{c}",
                                    tag="gp", bufs=4)

                    def gate_block(pstile, mb, wid):
                        nc.tensor.matmul(pstile[:, mb, :],
                                         wrz8[:, 0:2, ms(wid)],
                                         mh[c][:, 0:2, :], start=True,
                                         stop=False, perf_mode=DR)
                        nc.tensor.matmul(pstile[:, mb, :],
                                         wrz8[:, 2, ms(wid)],
                                         mh[c][:, 2, :], start=False,
                                         stop=False)
                        for k in range(2):
                            nc.tensor.matmul(pstile[:, mb, :],
                                             wrz16[:, k, ms(wid)],
                                             hprev[:, k, :], start=False,
                                             stop=(k == 1))

                    # r blocks first -> g01 ACT early
                    for mb in range(2):
                        gate_block(rr, mb, mb)
                    g01 = wp.tile([128, 2, CB], f16, name=f"g01_{t}_{c}",
                                  tag="g01", bufs=2)
                    nc.scalar.activation(g01[:], rr[:], AF.Tanh,
                                         scale=0.5 / S)
                    # i_n path: fp8 x-side + fp16 bias row (h16 slot2)
                    for mb in range(2):
                        nc.tensor.matmul(inp[:, mb, :], wp8[:, 0:2, ms(mb)],
                                         mh[c][:, 0:2, :], start=True,
                                         stop=False, perf_mode=DR)
                        nc.tensor.matmul(inp[:, mb, :], wp8[:, 2, ms(mb)],
                                         mh[c][:, 2, :], start=False,
                                         stop=False)
                        nc.tensor.matmul(inp[:, mb, :], wbin16[:, ms(mb)],
                                         hprev[:, 2, :], start=False,
                                         stop=True)
                    # z blocks (off the n-gate chain)
                    for mb in range(2):
                        gate_block(zz_ps, mb, 2 + mb)
                    g23 = wp.tile([128, 2, CB], f16, name=f"g23_{t}_{c}",
                                  tag="g23", bufs=2)
                    nc.scalar.activation(g23[:], zz_ps[:], AF.Tanh,
                                         scale=0.5 / S)

                    # rt = (g01+1)*t1h = S * r * (Whh_n.h + b_hh_n)
                    rt = wp.tile([128, 2, CB], f16, name=f"rt_{t}_{c}",
                                 tag="rt", bufs=2)
                    nc.vector.scalar_tensor_tensor(
                        rt[:], g01[:], 1.0, t1h[:], OP.add, OP.mult)
                    s2 = wp.tile([128, 2, CB], f16, name=f"s2_{t}_{c}",
                                 tag="s2", bufs=2)
                    nc.vector.tensor_tensor(out=s2[:], in0=inp[:],
                                            in1=rt[:], op=OP.add)
                    n16 = wp.tile([128, 2, CB], f16, name=f"n_{t}_{c}",
                                  tag="n", bufs=2)
                    nc.scalar.activation(n16[:], s2[:], AF.Tanh,
                                         scale=1.0 / S)
                    # z path: zz/w1z on DVE, bzh on gpsimd (all off-chain)
                    zz = wp.tile([128, 2, CB], f16, name=f"zz_{t}_{c}",
                                 tag="zz", bufs=2)
                    nc.vector.tensor_scalar(out=zz[:], in0=g23[:],
                                            scalar1=0.5, scalar2=0.5,
                                            op0=OP.mult, op1=OP.add)
                    w1z = wp.tile([128, 2, CB], f16, name=f"w1z_{t}_{c}",
                                  tag="w1z", bufs=2)
                    nc.vector.tensor_scalar(out=w1z[:], in0=g23[:],
                                            scalar1=-0.5, scalar2=0.5,
                                            op0=OP.mult, op1=OP.add)
                    bzh = wp.tile([128, 2, CB], f16, name=f"bzh_{t}_{c}",
                                  tag="bzh", bufs=2)
                    nc.gpsimd.tensor_tensor(out=bzh[:], in0=zz[:],
                                            in1=hprev[:, 0:2, :], op=OP.mult)
                    a4 = wp.tile([128, 2, CB], f16, name=f"a4_{t}_{c}",
                                 tag="a4", bufs=2)
                    nc.vector.tensor_tensor(out=a4[:], in0=w1z[:],
                                            in1=n16[:], op=OP.mult)
                    hnew = wp.tile([128, 3, CB], f16, name=f"h16_{t}_{c}",
                                   tag="h16", bufs=H_BUFS)
                    halloc = NCHUNK + t * NCHUNK + c
                    if halloc < H_BUFS:
                        nc.scalar.dma_start(out=hnew[:, 2, :], in_=h2c16d)
                    nc.vector.tensor_tensor(out=hnew[:, 0:2, :], in0=a4[:],
                                            in1=bzh[:], op=OP.add)
                    h16[c] = hnew
                    nc.sync.dma_start(out=outd[t, c], in_=hnew[:, 0:2, :])

                for c in range(NCHUNK):
                    if t + 1 < t_steps:
                        mh[c] = mh_nxt[c]
                        x16[c] = x16[NCHUNK + c]

    nc.compile()
    return nc


# ---------------- host-side data prep ----------------

def _prep_core_inputs(x, h0, attn1_W, attn1_b, attn2_W, attn2_b, attn3_W,
                      attn3_b, W_ih, b_ih, W_hh, b_hh, t_steps=T):
    import ml_dtypes
    f4 = np.float32
    f16n = np.float16
    f8n = ml_dtypes.float8_e4m3

    x = np.asarray(x, f4)
    h0 = np.asarray(h0, f4)

    A1T = np.zeros((H, IP), f4)
    A1T[:, :I] = np.asarray(attn1_W, f4).T
    A2T = np.zeros((IP, IP), f4)
    A2T[:I, :I] = np.asarray(attn2_W, f4).T
    A3T = np.zeros((IP, IP), f4)
    A3T[:I, :I] = np.asarray(attn3_W, f4).T
    WihT = np.zeros((IP, 3 * H), f4)
    WihT[:I, :] = np.asarray(W_ih, f4).T
    WhhT = np.asarray(W_hh, f4).T                      # [256, 768]
    bu = np.zeros(IP, f4)
    bu[:I] = np.asarray(attn1_b, f4) + np.asarray(attn2_b, f4)
    bv = np.full(IP, -448.0, f4)
    bv[:I] = np.asarray(attn3_b, f4)
    brz = (np.asarray(b_ih, f4) + np.asarray(b_hh, f4))[:2 * H]
    bhn = np.asarray(b_hh, f4)[2 * H:]
    bin_ = np.asarray(b_ih, f4)[2 * H:]

    # wu8: [128, 3, IP]: slots (x0, x1) for the DR pair, x2 plain (+bu row)
    wu = np.zeros((3, 128, IP), f4)
    wu[0] = A2T[0:128]
    wu[1] = A2T[128:256]
    wu[2] = A2T[256:384]
    wu[2, 64, :] = bu
    wu8 = np.ascontiguousarray(wu.transpose(1, 0, 2)).astype(f8n)

    # wu16: h-side of ps_u (attn1)
    wuh = np.stack([A1T[0:128], A1T[128:256]])
    wu16 = np.ascontiguousarray(wuh.transpose(1, 0, 2)).astype(f16n)

    # wv8: DR pairs (u0,u1),(u2,uz); uz row0 carries bv
    wv = np.zeros((2, 2, 128, IP), f4)
    wv[0, 0] = A3T[0:128]
    wv[0, 1] = A3T[128:256]
    wv[1, 0] = A3T[256:384]
    wv[1, 1, 0, :] = bv
    wv8 = np.ascontiguousarray(wv.transpose(2, 0, 1, 3)).astype(f8n)

    # wrz8: x-side, slots (x0,x1) DR + x2 (+S*brz row); wrz16: h-side *S
    wrz = np.zeros((3, 128, 512), f4)
    wrz[0] = WihT[0:128, :512]
    wrz[1] = WihT[128:256, :512]
    wrz[2] = WihT[256:384, :512]
    wrz[2, 64, :] = S * brz
    wrz8 = np.ascontiguousarray(wrz.transpose(1, 0, 2)).astype(f8n)
    wrzh = np.stack([S * WhhT[0:128, :512], S * WhhT[128:256, :512]])
    wrz16 = np.ascontiguousarray(wrzh.transpose(1, 0, 2)).astype(f16n)

    # wp8: i_n x-side (no bias row); wbin16: fp16 ones-row bias matmul
    wpp = np.zeros((3, 128, 256), f4)
    wpp[0] = WihT[0:128, 512:]
    wpp[1] = WihT[128:256, 512:]
    wpp[2] = WihT[256:384, 512:]
    wp8 = np.ascontiguousarray(wpp.transpose(1, 0, 2)).astype(f8n)
    wbin = np.zeros((128, 256), f4)
    wbin[0, :] = S * bin_
    wbin16 = wbin.astype(f16n)

    # wt16: t1h = S*0.5*(Whh_n.h + b_hh_n); k2 = fp16 ones-row bias
    wt = np.zeros((3, 128, 256), f4)
    wt[0] = 0.5 * S * WhhT[0:128, 512:]
    wt[1] = 0.5 * S * WhhT[128:256, 512:]
    wt[2, 0, :] = 0.5 * S * bhn
    wt16 = np.ascontiguousarray(wt.transpose(1, 0, 2)).astype(f16n)

    ones16 = np.full((128, 128), 1.0 / S, f16n)
    uz8 = np.zeros((128, CB), f8n)
    uz8[0, :] = 1.0
    h2c16 = np.zeros((128, CB), f16n)
    h2c16[0, :] = 1.0

    # x: pad to IP, plant the ones-row at feature 320 (slot2 row64)
    xp = np.zeros((B, t_steps, IP), f4)
    xp[:, :, :I] = x[:, :t_steps, :]
    xp[:, :, I] = 1.0
    xr = xp.reshape(NCORES, NCHUNK, CB, t_steps, KI, 128)
    xr = xr.transpose(0, 3, 1, 5, 4, 2)
    x8 = np.ascontiguousarray(xr).astype(f8n)
    x16 = np.ascontiguousarray(xr).astype(f16n)

    h0r = h0.reshape(NCORES, NCHUNK, CB, KH, 128).transpose(0, 1, 4, 3, 2)
    h016 = np.ascontiguousarray(h0r).astype(f16n)

    shared = dict(wu8=wu8, wv8=wv8, wrz8=wrz8, wp8=wp8, wu16=wu16,
                  wrz16=wrz16, wt16=wt16, wbin16=wbin16, ones16=ones16,
                  uz8=uz8, h2c16=h2c16)
    in_maps = []
    for c in range(NCORES):
        m = dict(shared)
        m["x8"] = x8[c]
        m["x16"] = x16[c]
        m["h016"] = h016[c]
        in_maps.append(m)
    return in_maps


def _gather(results, t_steps=T):
    outs = []
    for c in range(NCORES):
        o = np.asarray(results[c]["outT"], np.float32)
        o = o.transpose(1, 4, 0, 3, 2).reshape(BS, t_steps, H)
        outs.append(o)
    return np.ascontiguousarray(np.concatenate(outs, axis=0))


def _get_nc(t_steps=T):
    key = ("nc", t_steps)
    if key not in _STATE:
        _STATE[key] = _build(t_steps)
    return _STATE[key]


def run(inputs, trace=False, t_steps=T):
    from concourse.bass_utils import run_bass_kernel_spmd
    nc = _get_nc(t_steps)
    in_maps = _prep_core_inputs(t_steps=t_steps, **inputs)
    res = run_bass_kernel_spmd(nc, in_maps, list(range(NCORES)), trace=trace)
    return _gather(res.results, t_steps), res


def kernel(**inputs):
    out, _ = run(inputs, trace=False)
    return out


# revision 8
# speedup vs baseline: 1.6408x; 1.0936x over previous
"""Trainium2 Bass kernel for nn_Encoder_81595788689580.

Attention-gated GRU encoder: per time step
    w1 = h @ attn1_W.T + attn1_b
    w2 = x_t @ attn2_W.T + attn2_b
    v  = tanh(w1 + w2) @ attn3_W.T + attn3_b
    alpha = softmax(v, axis=feature)
    wx = x_t * alpha
    GRU cell (r, z, n) -> h_new
Output: [B, T, H] hidden states.

Strategy (8 NeuronCores, data-parallel over batch; 512 rows/core run as
2 pipelined chunks of 256):
  - transposed layout: features on partitions, batch on the free dim;
    weights-stationary matmuls, biases ride inside the matmuls as
    ones-rows planted in the zero-padding (x feature 320 = 1.0, and an
    fp16 ones-row slot in the h state tile).
  - x-side contractions in fp8(e4m3): the (x0,x1) 256-row pair runs as
    one DoubleRow matmul (2x), x2 as a plain fp8 matmul carrying the
    bias row; h-side contractions stay fp16 (exact recurrent path,
    keeps PE busy enough to hold the 2.4GHz p-state, and avoids an
    h->fp8 cast on the critical path).
  - wx is stored *S (S=16) in fp8 (normal range), descaled by the ACT
    `scale` at each gate activation; the r/z h-side weights carry S,
    the n-path h-side weights carry 0.5*S (folding the sigmoid scale).
  - precision: n-gate path fp16 end-to-end; sim rel err ~2e-3.
  - chain shortening: h-only matmuls (t1h, bias rows) fill the PE
    while ACT runs; the gate ACT is split r-first/z-second so the
    n-gate chain starts after only the r blocks; the z-path products
    (zz, w1z on DVE, bzh on gpsimd) run off the critical chain.
  - softmax denominator: fp16 ones-matmul (value 1/S) so reciprocal
    yields the S-scaled normalizer directly (read as f32 by the DVE
    alpha multiply; no cast hop).
"""

import numpy as np

B, T, I, H = 4096, 24, 320, 256
NCORES = 8
BS = B // NCORES          # 512 rows per core
NCHUNK = 2
CB = BS // NCHUNK         # 256 batch columns per chunk
IP = 384                  # I padded to 3*128
KI = IP // 128            # 3 feature blocks
KH = H // 128             # 2 hidden blocks
S = 16.0                  # wx / gate-psum scale (fp8 range)

_STATE = {}


def _build(t_steps=T):
    import concourse.bass as bass
    import concourse.tile as tile
    from concourse import bacc, mybir

    f32 = mybir.dt.float32
    f16 = mybir.dt.float16
    f8 = mybir.dt.float8e4
    AF = mybir.ActivationFunctionType
    OP = mybir.AluOpType
    DR = mybir.MatmulPerfMode.DoubleRow

    nc = bacc.Bacc("TRN2", target_bir_lowering=False, debug=False,
                   num_devices=NCORES)

    x8d = nc.dram_tensor("x8", [t_steps, NCHUNK, 128, KI, CB], f8,
                         kind="ExternalInput").ap()
    x16d = nc.dram_tensor("x16", [t_steps, NCHUNK, 128, KI, CB], f16,
                          kind="ExternalInput").ap()
    h016d = nc.dram_tensor("h016", [NCHUNK, 128, KH, CB], f16,
                           kind="ExternalInput").ap()
    wu8d = nc.dram_tensor("wu8", [128, 3, IP], f8,
                          kind="ExternalInput").ap()
    wv8d = nc.dram_tensor("wv8", [128, 2, 2, IP], f8,
                          kind="ExternalInput").ap()
    wrz8d = nc.dram_tensor("wrz8", [128, 3, 512], f8,
                           kind="ExternalInput").ap()
    wp8d = nc.dram_tensor("wp8", [128, 3, 256], f8,
                          kind="ExternalInput").ap()
    wu16d = nc.dram_tensor("wu16", [128, 2, IP], f16,
                           kind="ExternalInput").ap()
    wrz16d = nc.dram_tensor("wrz16", [128, 2, 512], f16,
                            kind="ExternalInput").ap()
    wt16d = nc.dram_tensor("wt16", [128, 3, 256], f16,
                           kind="ExternalInput").ap()
    wbin16d = nc.dram_tensor("wbin16", [128, 256], f16,
                             kind="ExternalInput").ap()
    ones16d = nc.dram_tensor("ones16", [128, 128], f16,
                             kind="ExternalInput").ap()
    uz8d = nc.dram_tensor("uz8", [128, CB], f8, kind="ExternalInput").ap()
    h2c16d = nc.dram_tensor("h2c16", [128, CB], f16,
                            kind="ExternalInput").ap()
    outd = nc.dram_tensor("outT", [t_steps, NCHUNK, 128, KH, CB], f16,
                          kind="ExternalOutput").ap()

    MH_BUFS = 4
    U_BUFS = 3
    H_BUFS = 4

    def ms(m):
        return slice(m * 128, (m + 1) * 128)

    with tile.TileContext(nc) as tc:
        with tc.tile_pool(name="const", bufs=1) as cp, \
             tc.tile_pool(name="mhp", bufs=1) as mp, \
             tc.tile_pool(name="wk", bufs=1) as wp, \
             tc.tile_pool(name="ps", bufs=1, space="PSUM") as pp:

            wu8 = cp.tile([128, 3, IP], f8)
            wv8 = cp.tile([128, 2, 2, IP], f8)
            wrz8 = cp.tile([128, 3, 512], f8)
            wp8 = cp.tile([128, 3, 256], f8)
            wu16 = cp.tile([128, 2, IP], f16)
            wrz16 = cp.tile([128, 2, 512], f16)
            wt16 = cp.tile([128, 3, 256], f16)
            wbin16 = cp.tile([128, 256], f16)
            ones16 = cp.tile([128, 128], f16)
            for i, (dst, src) in enumerate([
                    (wu8, wu8d), (wv8, wv8d), (wrz8, wrz8d), (wp8, wp8d),
                    (wu16, wu16d), (wrz16, wrz16d), (wt16, wt16d),
                    (wbin16, wbin16d), (ones16, ones16d)]):
                eng = nc.sync if i % 2 == 0 else nc.scalar
                eng.dma_start(out=dst[:], in_=src)

            mh = {}
            h16 = {}
            x16 = {}
            u_open = {}
            for c in range(NCHUNK):
                mt = mp.tile([128, KI, CB], f8, name=f"mh_0_{c}", tag="mh",
                             bufs=MH_BUFS)
                nc.sync.dma_start(out=mt[:], in_=x8d[0, c])
                mh[c] = mt
                ht = wp.tile([128, 3, CB], f16, name=f"h16_i_{c}", tag="h16",
                             bufs=H_BUFS)
                nc.scalar.dma_start(out=ht[:, 0:2, :], in_=h016d[c])
                nc.scalar.dma_start(out=ht[:, 2, :], in_=h2c16d)
                h16[c] = ht
                xt = wp.tile([128, KI, CB], f16, name=f"x16_0_{c}", tag="x16",
                             bufs=4)
                nc.sync.dma_start(out=xt[:], in_=x16d[0, c])
                x16[c] = xt

            def open_u(t, c):
                # pre-open the ps_u groups with the x-side matmuls;
                # the h-side lands in phase A once h(t-1) exists
                att_u = pp.tile([128, 4, CB], f32, name=f"psu_{t}_{c}",
                                tag="att", bufs=2)
                for m in range(3):
                    nc.tensor.matmul(att_u[:, m, :], wu8[:, 0:2, ms(m)],
                                     mh[c][:, 0:2, :], start=True,
                                     stop=False, perf_mode=DR)
                    nc.tensor.matmul(att_u[:, m, :], wu8[:, 2, ms(m)],
                                     mh[c][:, 2, :], start=False,
                                     stop=False)
                u_open[c] = att_u

            for c in range(NCHUNK):
                open_u(0, c)

            for t in range(t_steps):
                mh_nxt = {}
                if t + 1 < t_steps:
                    for c in range(NCHUNK):
                        mt = mp.tile([128, KI, CB], f8, name=f"mh_{t + 1}_{c}",
                                     tag="mh", bufs=MH_BUFS)
                        nc.sync.dma_start(out=mt[:], in_=x8d[t + 1, c])
                        mh_nxt[c] = mt
                        xt = wp.tile([128, KI, CB], f16,
                                     name=f"x16_{t + 1}_{c}", tag="x16",
                                     bufs=4)
                        nc.sync.dma_start(out=xt[:], in_=x16d[t + 1, c])
                        x16[NCHUNK + c] = xt

                st = [{} for _ in range(NCHUNK)]

                # ---- phase A: finish ps_u with the h-side, tanh ----
                for c in range(NCHUNK):
                    att_u = u_open[c]
                    for m in range(3):
                        for k in range(2):
                            nc.tensor.matmul(att_u[:, m, :],
                                             wu16[:, k, ms(m)],
                                             h16[c][:, k, :], start=False,
                                             stop=(k == 1))
                    st[c]["att_u"] = att_u

                # t1h(c0) (h-only, fp16): fills PE while tanh/exp run
                def emit_t1h(c):
                    t1h = pp.tile([128, 2, CB], f32, name=f"t1h_{t}_{c}",
                                  tag="gp", bufs=4)
                    for mb in range(2):
                        for k in range(3):
                            nc.tensor.matmul(
                                t1h[:, mb, :], wt16[:, k, ms(mb)],
                                h16[c][:, k, :], start=(k == 0),
                                stop=(k == 2))
                    st[c]["t1h"] = t1h

                emit_t1h(0)

                for c in range(NCHUNK):
                    u8 = wp.tile([128, 4, CB], f8, name=f"u8_{t}_{c}",
                                 tag="u8", bufs=U_BUFS)
                    if t * NCHUNK + c < U_BUFS:
                        nc.scalar.dma_start(out=u8[:, 3, :], in_=uz8d)
                    nc.scalar.activation(u8[:, 0:2, :],
                                         st[c]["att_u"][:, 0:2, :], AF.Tanh)
                    nc.scalar.activation(u8[:, 2, :],
                                         st[c]["att_u"][:, 2, :], AF.Tanh)
                    st[c]["u8"] = u8

                # ---- phase B: ps_v = attn3.u (+bv), exp (split) ----
                for c in range(NCHUNK):
                    att_v = pp.tile([128, 4, CB], f32, name=f"psv_{t}_{c}",
                                    tag="att", bufs=2)
                    u8 = st[c]["u8"]
                    for m in range(3):
                        for p in range(2):
                            nc.tensor.matmul(
                                att_v[:, m, :], wv8[:, p, :, ms(m)],
                                u8[:, 2 * p:2 * p + 2, :],
                                start=(p == 0), stop=(p == 1), perf_mode=DR)
                    ev = wp.tile([128, 3, CB], f16, name=f"ev_{t}_{c}",
                                 tag="ev", bufs=3)
                    nc.scalar.activation(ev[:, 0:2, :], att_v[:, 0:2, :],
                                         AF.Exp)
                    nc.scalar.activation(ev[:, 2, :], att_v[:, 2, :], AF.Exp)
                    st[c]["ev"] = ev
                    st[c]["att_v"] = att_v

                # ---- phase C: den, xev, rinv, wx ----
                for c in range(NCHUNK):
                    ev = st[c]["ev"]
                    att_v = st[c]["att_v"]
                    for k in range(3):
                        nc.tensor.matmul(att_v[:, 3, :], ones16[:],
                                         ev[:, k, :], start=(k == 0),
                                         stop=(k == 2))
                    xc = x16[c]
                    xev = wp.tile([128, 3, CB], f16, name=f"xev_{t}_{c}",
                                  tag="xev", bufs=2)
                    nc.vector.tensor_tensor(out=xev[:, 0:2, :],
                                            in0=xc[:, 0:2, :],
                                            in1=ev[:, 0:2, :], op=OP.mult)
                    nc.gpsimd.tensor_tensor(out=xev[0:64, 2, :],
                                            in0=xc[0:64, 2, :],
                                            in1=ev[0:64, 2, :], op=OP.mult)
                    rinv = wp.tile([128, CB], f32, name=f"rinv_{t}_{c}",
                                   tag="rinv", bufs=2)
                    nc.vector.reciprocal_approx_fast(rinv[:], att_v[:, 3, :])
                    _r = rinv[:]
                    rrep = bass.AP(tensor=_r.tensor, offset=_r.offset,
                                   ap=[_r.ap[0], [0, 2], _r.ap[1]])
                    nc.vector.tensor_tensor(out=mh[c][:, 0:2, :],
                                            in0=xev[:, 0:2, :], in1=rrep,
                                            op=OP.mult)
                    nc.gpsimd.tensor_tensor(out=mh[c][0:64, 2, :],
                                            in0=xev[0:64, 2, :],
                                            in1=rinv[0:64, :], op=OP.mult)
                    # pre-open next step's ps_u groups (x-only) to keep
                    # the PE busy while wx lands
                    if t + 1 < t_steps:
                        sv_mh = mh[c]
                        mh[c] = mh_nxt[c]
                        open_u(t + 1, c)
                        mh[c] = sv_mh

                # ---- phase D: gates + GRU tail ----
                for c in range(NCHUNK):
                    hprev = h16[c]
                    if c == 1:
                        emit_t1h(1)
                    t1h = st[c]["t1h"]
                    rr = pp.tile([128, 2, CB], f32, name=f"rps_{t}_{c}",
                                 tag="gp", bufs=4)
                    inp = pp.tile([128, 2, CB], f32, name=f"inps_{t}_{c}",
                                  tag="gp", bufs=4)
                    zz_ps = pp.tile([128, 2, CB], f32, name=f"zps_{t}_{c}",
                                    tag="gp", bufs=4)

                    def gate_block(pstile, mb, wid):
                        nc.tensor.matmul(pstile[:, mb, :],
                                         wrz8[:, 0:2, ms(wid)],
                                         mh[c][:, 0:2, :], start=True,
                                         stop=False, perf_mode=DR)
                        for k in range(2):
                            nc.tensor.matmul(pstile[:, mb, :],
                                             wrz16[:, k, ms(wid)],
                                             hprev[:, k, :], start=False,
                                             stop=False)
                        nc.tensor.matmul(pstile[:, mb, :],
                                         wrz8[:, 2, ms(wid)],
                                         mh[c][:, 2, :], start=False,
                                         stop=True)

                    for mb in range(2):
                        gate_block(rr, mb, mb)
                    g01 = wp.tile([128, 2, CB], f16, name=f"g01_{t}_{c}",
                                  tag="g01", bufs=2)
                    nc.scalar.activation(g01[:], rr[:], AF.Tanh,
                                         scale=0.5 / S)
                    for mb in range(2):
                        nc.tensor.matmul(inp[:, mb, :], wp8[:, 0:2, ms(mb)],
                                         mh[c][:, 0:2, :], start=True,
                                         stop=False, perf_mode=DR)
                        nc.tensor.matmul(inp[:, mb, :], wbin16[:, ms(mb)],
                                         hprev[:, 2, :], start=False,
                                         stop=False)
                        nc.tensor.matmul(inp[:, mb, :], wp8[:, 2, ms(mb)],
                                         mh[c][:, 2, :], start=False,
                                         stop=True)
                    for mb in range(2):
                        gate_block(zz_ps, mb, 2 + mb)
                    g23 = wp.tile([128, 2, CB], f16, name=f"g23_{t}_{c}",
                                  tag="g23", bufs=2)
                    nc.scalar.activation(g23[:], zz_ps[:], AF.Tanh,
                                         scale=0.5 / S)

                    rt = wp.tile([128, 2, CB], f16, name=f"rt_{t}_{c}",
                                 tag="rt", bufs=2)
                    nc.vector.scalar_tensor_tensor(
                        rt[:], g01[:], 1.0, t1h[:], OP.add, OP.mult)
                    s2 = wp.tile([128, 2, CB], f16, name=f"s2_{t}_{c}",
                                 tag="s2", bufs=2)
                    nc.vector.tensor_tensor(out=s2[:], in0=inp[:],
                                            in1=rt[:], op=OP.add)
                    n16 = wp.tile([128, 2, CB], f16, name=f"n_{t}_{c}",
                                  tag="n", bufs=2)
                    nc.scalar.activation(n16[:], s2[:], AF.Tanh,
                                         scale=1.0 / S)
                    zz = wp.tile([128, 2, CB], f16, name=f"zz_{t}_{c}",
                                 tag="zz", bufs=2)
                    nc.gpsimd.tensor_scalar(out=zz[:], in0=g23[:],
                                            scalar1=0.5, scalar2=0.5,
                                            op0=OP.mult, op1=OP.add)
                    w1z = wp.tile([128, 2, CB], f16, name=f"w1z_{t}_{c}",
                                  tag="w1z", bufs=2)
                    nc.vector.tensor_scalar(out=w1z[:], in0=g23[:],
                                            scalar1=-0.5, scalar2=0.5,
                                            op0=OP.mult, op1=OP.add)
                    bzh = wp.tile([128, 2, CB], f16, name=f"bzh_{t}_{c}",
                                  tag="bzh", bufs=2)
                    nc.vector.tensor_tensor(out=bzh[:], in0=zz[:],
                                            in1=hprev[:, 0:2, :], op=OP.mult)
                    a4 = wp.tile([128, 2, CB], f16, name=f"a4_{t}_{c}",
                                 tag="a4", bufs=2)
                    nc.vector.tensor_tensor(out=a4[:], in0=w1z[:],
                                            in1=n16[:], op=OP.mult)
                    hnew = wp.tile([128, 3, CB], f16, name=f"h16_{t}_{c}",
                                   tag="h16", bufs=H_BUFS)
                    halloc = NCHUNK + t * NCHUNK + c
                    if halloc < H_BUFS:
                        nc.scalar.dma_start(out=hnew[:, 2, :], in_=h2c16d)
                    nc.vector.tensor_tensor(out=hnew[:, 0:2, :], in0=a4[:],
                                            in1=bzh[:], op=OP.add)
                    h16[c] = hnew
                    nc.sync.dma_start(out=outd[t, c], in_=hnew[:, 0:2, :])

                for c in range(NCHUNK):
                    if t + 1 < t_steps:
                        mh[c] = mh_nxt[c]
                        x16[c] = x16[NCHUNK + c]

    nc.compile()
    return nc


# ---------------- host-side data prep ----------------

def _prep_core_inputs(x, h0, attn1_W, attn1_b, attn2_W, attn2_b, attn3_W,
                      attn3_b, W_ih, b_ih, W_hh, b_hh, t_steps=T):
    import ml_dtypes
    f4 = np.float32
    f16n = np.float16
    f8n = ml_dtypes.float8_e4m3

    x = np.asarray(x, f4)
    h0 = np.asarray(h0, f4)

    A1T = np.zeros((H, IP), f4)
    A1T[:, :I] = np.asarray(attn1_W, f4).T
    A2T = np.zeros((IP, IP), f4)
    A2T[:I, :I] = np.asarray(attn2_W, f4).T
    A3T = np.zeros((IP, IP), f4)
    A3T[:I, :I] = np.asarray(attn3_W, f4).T
    WihT = np.zeros((IP, 3 * H), f4)
    WihT[:I, :] = np.asarray(W_ih, f4).T
    WhhT = np.asarray(W_hh, f4).T                      # [256, 768]
    bu = np.zeros(IP, f4)
    bu[:I] = np.asarray(attn1_b, f4) + np.asarray(attn2_b, f4)
    bv = np.full(IP, -448.0, f4)
    bv[:I] = np.asarray(attn3_b, f4)
    brz = (np.asarray(b_ih, f4) + np.asarray(b_hh, f4))[:2 * H]
    bhn = np.asarray(b_hh, f4)[2 * H:]
    bin_ = np.asarray(b_ih, f4)[2 * H:]

    # wu8: [128, 3, IP]: slots (x0, x1) for the DR pair, x2 plain (+bu row)
    wu = np.zeros((3, 128, IP), f4)
    wu[0] = A2T[0:128]
    wu[1] = A2T[128:256]
    wu[2] = A2T[256:384]
    wu[2, 64, :] = bu
    wu8 = np.ascontiguousarray(wu.transpose(1, 0, 2)).astype(f8n)

    # wu16: h-side of ps_u (attn1)
    wuh = np.stack([A1T[0:128], A1T[128:256]])
    wu16 = np.ascontiguousarray(wuh.transpose(1, 0, 2)).astype(f16n)

    # wv8: DR pairs (u0,u1),(u2,uz); uz row0 carries bv
    wv = np.zeros((2, 2, 128, IP), f4)
    wv[0, 0] = A3T[0:128]
    wv[0, 1] = A3T[128:256]
    wv[1, 0] = A3T[256:384]
    wv[1, 1, 0, :] = bv
    wv8 = np.ascontiguousarray(wv.transpose(2, 0, 1, 3)).astype(f8n)

    # wrz8: x-side, slots (x0,x1) DR + x2 (+S*brz row); wrz16: h-side *S
    wrz = np.zeros((3, 128, 512), f4)
    wrz[0] = WihT[0:128, :512]
    wrz[1] = WihT[128:256, :512]
    wrz[2] = WihT[256:384, :512]
    wrz[2, 64, :] = S * brz
    wrz8 = np.ascontiguousarray(wrz.transpose(1, 0, 2)).astype(f8n)
    wrzh = np.stack([S * WhhT[0:128, :512], S * WhhT[128:256, :512]])
    wrz16 = np.ascontiguousarray(wrzh.transpose(1, 0, 2)).astype(f16n)

    # wp8: i_n x-side (no bias row); wbin16: fp16 ones-row bias matmul
    wpp = np.zeros((3, 128, 256), f4)
    wpp[0] = WihT[0:128, 512:]
    wpp[1] = WihT[128:256, 512:]
    wpp[2] = WihT[256:384, 512:]
    wp8 = np.ascontiguousarray(wpp.transpose(1, 0, 2)).astype(f8n)
    wbin = np.zeros((128, 256), f4)
    wbin[0, :] = S * bin_
    wbin16 = wbin.astype(f16n)

    # wt16: t1h = S*0.5*(Whh_n.h + b_hh_n); k2 = fp16 ones-row bias
    wt = np.zeros((3, 128, 256), f4)
    wt[0] = 0.5 * S * WhhT[0:128, 512:]
    wt[1] = 0.5 * S * WhhT[128:256, 512:]
    wt[2, 0, :] = 0.5 * S * bhn
    wt16 = np.ascontiguousarray(wt.transpose(1, 0, 2)).astype(f16n)

    ones16 = np.full((128, 128), 1.0 / S, f16n)
    uz8 = np.zeros((128, CB), f8n)
    uz8[0, :] = 1.0
    h2c16 = np.zeros((128, CB), f16n)
    h2c16[0, :] = 1.0

    # x: pad to IP, plant the ones-row at feature 320 (slot2 row64)
    xp = np.zeros((B, t_steps, IP), f4)
    xp[:, :, :I] = x[:, :t_steps, :]
    xp[:, :, I] = 1.0
    xr = xp.reshape(NCORES, NCHUNK, CB, t_steps, KI, 128)
    xr = xr.transpose(0, 3, 1, 5, 4, 2)
    x8 = np.ascontiguousarray(xr).astype(f8n)
    x16 = np.ascontiguousarray(xr).astype(f16n)

    h0r = h0.reshape(NCORES, NCHUNK, CB, KH, 128).transpose(0, 1, 4, 3, 2)
    h016 = np.ascontiguousarray(h0r).astype(f16n)

    shared = dict(wu8=wu8, wv8=wv8, wrz8=wrz8, wp8=wp8, wu16=wu16,
                  wrz16=wrz16, wt16=wt16, wbin16=wbin16, ones16=ones16,
                  uz8=uz8, h2c16=h2c16)
    in_maps = []
    for c in range(NCORES):
        m = dict(shared)
        m["x8"] = x8[c]
        m["x16"] = x16[c]
        m["h016"] = h016[c]
        in_maps.append(m)
    return in_maps


def _gather(results, t_steps=T):
    outs = []
    for c in range(NCORES):
        o = np.asarray(results[c]["outT"], np.float32)
        o = o.transpose(1, 4, 0, 3, 2).reshape(BS, t_steps, H)
        outs.append(o)
    return np.ascontiguousarray(np.concatenate(outs, axis=0))


def _get_nc(t_steps=T):
    key = ("nc", t_steps)
    if key not in _STATE:
        _STATE[key] = _build(t_steps)
    return _STATE[key]


def run(inputs, trace=False, t_steps=T):
    from concourse.bass_utils import run_bass_kernel_spmd
    nc = _get_nc(t_steps)
    in_maps = _prep_core_inputs(t_steps=t_steps, **inputs)
    res = run_bass_kernel_spmd(nc, in_maps, list(range(NCORES)), trace=trace)
    return _gather(res.results, t_steps), res


def kernel(**inputs):
    out, _ = run(inputs, trace=False)
    return out
